# revision 20
# baseline (speedup 1.0000x reference)
"""Trainium2 Bass kernel for nn_SmartCNN (2048-style board CNN policy net).

Algorithm (per sample, 4x4 board of exponents e in [0,11)):
  1. flips (fv, fh) from corner argmax of the board (first-max-wins),
     applied as data flips to the int exponents (sample-major, DVE selects).
  2. one-hot(176) of flipped exponents built feature-major:
     bf16 cast -> DMA-xbar transpose -> DMA partition-replication x11 ->
     is_equal against per-partition class id.
  3. z1 = relu(G @ oh + b1)   [96]   (G fuses both convs + the constant
     mask channel; computed on host from conv weights)
  4. z2 = relu(W1 @ z1 + b2)  [256]
  5. Lg = W2x4 @ z2 + b3x4    [16]   (4 flip-variant-permuted copies of the
     output layer -> per-sample variant selected after softmax transpose)
  6. out = softmax(Lg[4f:4f+4]), f = 2*fv+fh; computed sample-major after a
     DMA transpose of exp(Lg - SHIFT).

Data parallel over 8 NeuronCores: each core does 32768 samples.
"""
import numpy as np
import ml_dtypes

B_TOTAL = 262144
N_CORES = 8
S = B_TOTAL // N_CORES          # 32768 samples per core
BIG = 8192                      # samples per big tile (sample-major stages)
NBIG = S // BIG                 # 4
NCHUNK = BIG // 1024            # 8 transpose chunks per big tile
CLASSES = 11

_COMPILED = None  # cached (nc, const_inputs) across calls


# ----------------------------------------------------------------- host math
def _build_weights(conv0_w, conv0_b, conv1_w, linear_w, linear_b, out_w, out_b):
    G = np.zeros((96, 176), np.float32)
    b1 = np.zeros((96,), np.float32)
    for oc in range(4):
        for i in range(4):
            for jj in range(3):
                k = oc * 12 + i * 3 + jj
                b1[k] = conv0_b[oc] + conv0_w[oc, 0, 0, 0] + conv0_w[oc, 0, 0, 1]
                for kx in range(2):
                    c = i * 4 + (jj + kx)
                    G[k, c * 11:(c + 1) * 11] += conv0_w[oc, 1:, 0, kx]
    for oc in range(4):
        for ii in range(3):
            for j in range(4):
                k = 48 + oc * 12 + ii * 4 + j
                b1[k] = conv1_w[oc, 0, 0, 0] + conv1_w[oc, 0, 1, 0]
                for ky in range(2):
                    c = (ii + ky) * 4 + j
                    G[k, c * 11:(c + 1) * 11] += conv1_w[oc, 1:, ky, 0]
    W2x4 = np.zeros((16, 256), np.float32)
    b3x4 = np.zeros((16,), np.float32)
    for fv in range(2):
        for fh in range(2):
            f = 2 * fv + fh
            perm = [1 if fv else 0, 0 if fv else 1, 3 if fh else 2, 2 if fh else 3]
            for j in range(4):
                W2x4[4 * f + j] = out_w[perm[j]]
                b3x4[4 * f + j] = out_b[perm[j]]
    # softmax shift: tight data-independent upper bound on the logits
    z1_hi = np.maximum(
        b1 + G.reshape(96, 16, 11).max(2).sum(1) - 0.0, 0)  # only 2 cells/k are nonzero; sum of per-cell maxes is a valid bound
    z2_hi = np.maximum(linear_b + np.maximum(linear_w, 0) @ z1_hi, 0)
    lg_hi = (b3x4 + np.maximum(W2x4, 0) @ z2_hi).max()
    shift = float(lg_hi) - 8.0
    return (G.astype(np.float32), b1.astype(np.float32),
            np.asarray(linear_w, np.float32), np.asarray(linear_b, np.float32),
            W2x4, b3x4, shift)


def _make_consts(inputs):
    G, b1, W1, b2, W2x4, b3x4, shift = _build_weights(
        inputs["conv0_w"], inputs["conv0_b"], inputs["conv1_w"],
        inputs["linear_w"], inputs["linear_b"], inputs["out_w"], inputs["out_b"])
    bf = ml_dtypes.bfloat16
    return {
        "G0c": np.ascontiguousarray(G[:, 0:88].T).astype(bf),       # [88,96]
        "G1c": np.ascontiguousarray(G[:, 88:176].T).astype(bf),     # [88,96]
        "b1col": b1.reshape(96, 1),
        "W1a": np.ascontiguousarray(W1[0:128].T).astype(bf),        # [96,128]
        "W1b": np.ascontiguousarray(W1[128:256].T).astype(bf),      # [96,128]
        "b2a": b2[0:128].reshape(128, 1).astype(np.float32),
        "b2b": b2[128:256].reshape(128, 1).astype(np.float32),
        "W2a": np.ascontiguousarray(W2x4[:, 0:128].T).astype(bf),   # [128,16]
        "W2b": np.ascontiguousarray(W2x4[:, 128:256].T).astype(bf), # [128,16]
        "Vcol": (np.arange(88, dtype=np.float32) % 11).reshape(88, 1),
        # sample-major softmax: constant part of the logit bias as an
        # all-equal [128,1] exp-bias column; per-column remainder as a
        # broadcast multiplier tile (all-ones when out_b is constant).
        "bshift": np.full((128, 1), b3x4[0] - shift, np.float32),
        "EB64": np.tile(np.exp(b3x4 - b3x4[0]).astype(np.float32), (128, 4)),
        "RepAll": _rep_matrices(),
    }


def _rep_matrices():
    """16 stationary one-hot broadcasters as column blocks of [128, 1408]:
    block i = (qt, half): (Rep_i.T @ Y)[(c,v), p] = Y[16qt + 8*half + c, p]."""
    rep = np.zeros((128, 16, 88), np.float32)
    for qt in range(8):
        for half in range(2):
            for c in range(8):
                for v in range(11):
                    rep[16 * qt + 8 * half + c, 2 * qt + half, c * 11 + v] = 1.0
    return rep.reshape(128, 1408).astype(ml_dtypes.bfloat16)


# ----------------------------------------------------------------- device IR
def build_ir(nc, samples=S):
    import concourse.mybir as mybir
    import concourse.tile as tile

    F32, I32 = mybir.dt.float32, mybir.dt.int32
    BF16, F16 = mybir.dt.bfloat16, mybir.dt.float16
    AOP = mybir.AluOpType
    AF = mybir.ActivationFunctionType

    nbig = samples // BIG

    ex = nc.dram_tensor("exponents", [samples, 16], I32, kind="ExternalInput")
    out = nc.dram_tensor("out", [samples, 4], F32, kind="ExternalOutput")
    cG0 = nc.dram_tensor("G0c", [88, 96], BF16, kind="ExternalInput")
    cG1 = nc.dram_tensor("G1c", [88, 96], BF16, kind="ExternalInput")
    cb1 = nc.dram_tensor("b1col", [96, 1], F32, kind="ExternalInput")
    cW1a = nc.dram_tensor("W1a", [96, 128], BF16, kind="ExternalInput")
    cW1b = nc.dram_tensor("W1b", [96, 128], BF16, kind="ExternalInput")
    cb2a = nc.dram_tensor("b2a", [128, 1], F32, kind="ExternalInput")
    cb2b = nc.dram_tensor("b2b", [128, 1], F32, kind="ExternalInput")
    cW2a = nc.dram_tensor("W2a", [128, 16], BF16, kind="ExternalInput")
    cW2b = nc.dram_tensor("W2b", [128, 16], BF16, kind="ExternalInput")
    cV = nc.dram_tensor("Vcol", [88, 1], F32, kind="ExternalInput")
    cbs = nc.dram_tensor("bshift", [128, 1], F32, kind="ExternalInput")
    cEB = nc.dram_tensor("EB64", [128, 64], F32, kind="ExternalInput")
    cRep = nc.dram_tensor("RepAll", [128, 1408], BF16, kind="ExternalInput")

    with tile.TileContext(nc) as tc:
        with (
            tc.tile_pool(name="const", bufs=1) as cpool,
            tc.tile_pool(name="big", bufs=2) as bpool,
            tc.tile_pool(name="chunk", bufs=3) as kpool,
            tc.tile_pool(name="mid", bufs=3) as mpool,
            tc.tile_pool(name="ps", bufs=1, space="PSUM") as ppool,
            tc.tile_pool(name="ps2", bufs=2, space="PSUM") as ppool2,
        ):
            # ---- load constants once
            G0 = cpool.tile([88, 96], BF16, tag="G0")
            G1 = cpool.tile([88, 96], BF16, tag="G1")
            b1c = cpool.tile([96, 1], F32, tag="b1c")
            W1a = cpool.tile([96, 128], BF16, tag="W1a")
            W1b = cpool.tile([96, 128], BF16, tag="W1b")
            b2ac = cpool.tile([128, 1], F32, tag="b2a")
            b2bc = cpool.tile([128, 1], F32, tag="b2b")
            W2a = cpool.tile([128, 16], BF16, tag="W2a")
            W2b = cpool.tile([128, 16], BF16, tag="W2b")
            Vc = cpool.tile([88, 1], F32, tag="Vc")
            bsc = cpool.tile([128, 1], F32, tag="bsc")
            EBc = cpool.tile([128, 64], F32, tag="EBc")
            Rep = cpool.tile([128, 1408], BF16, tag="Rep")
            for t, d in ((G0, cG0), (G1, cG1), (b1c, cb1), (W1a, cW1a),
                         (W1b, cW1b), (b2ac, cb2a), (b2bc, cb2b), (W2a, cW2a),
                         (W2b, cW2b), (Vc, cV), (bsc, cbs), (EBc, cEB),
                         (Rep, cRep)):
                nc.sync.dma_start(out=t[:], in_=d[:])

            exv = ex[:].rearrange("(n p q) c -> n p (q c)", p=128, q=64)
            outv = out[:].rearrange("(n p q) j -> n p (q j)", p=128, q=64)

            for n in range(nbig):
                # ============================== stage A: flips (sample-major)
                X = bpool.tile([128, 1024], I32, tag="X")
                nc.sync.dma_start(out=X[:], in_=exv[n])
                X4 = X[:].rearrange("p (q i j) -> p q i j", i=4, j=4)
                c0, c3 = X4[:, :, 0, 0], X4[:, :, 0, 3]
                c12, c15 = X4[:, :, 3, 0], X4[:, :, 3, 3]
                m01 = bpool.tile([128, 64], I32, tag="m01")
                m23 = bpool.tile([128, 64], I32, tag="m23")
                FV = bpool.tile([128, 64], I32, tag="FV")
                FH = bpool.tile([128, 64], I32, tag="FH")
                H1 = bpool.tile([128, 64], I32, tag="H1")
                nc.vector.tensor_tensor(out=m01[:], in0=c0, in1=c3, op=AOP.max)
                nc.vector.tensor_tensor(out=m23[:], in0=c12, in1=c15, op=AOP.max)
                nc.vector.tensor_tensor(out=FV[:], in0=m23[:], in1=m01[:], op=AOP.is_gt)
                nc.vector.tensor_tensor(out=FH[:], in0=c3, in1=c0, op=AOP.is_gt)
                nc.vector.tensor_tensor(out=H1[:], in0=c15, in1=c12, op=AOP.is_gt)
                nc.vector.copy_predicated(FH[:], FV[:], H1[:])

                MV = bpool.tile([128, 1024], I32, tag="MV")
                MH = bpool.tile([128, 1024], I32, tag="MH")
                nc.gpsimd.tensor_copy(
                    out=MV[:].rearrange("p (q c) -> p q c", c=16),
                    in_=FV[:].unsqueeze(2).broadcast_to([128, 64, 16]))
                nc.gpsimd.tensor_copy(
                    out=MH[:].rearrange("p (q c) -> p q c", c=16),
                    in_=FH[:].unsqueeze(2).broadcast_to([128, 64, 16]))

                Bv = bpool.tile([128, 1024], I32, tag="Bv")
                Ch = bpool.tile([128, 1024], I32, tag="Ch")
                Bv4 = Bv[:].rearrange("p (q i j) -> p q i j", i=4, j=4)
                Ch4 = Ch[:].rearrange("p (q i j) -> p q i j", i=4, j=4)
                nc.gpsimd.tensor_copy(out=Bv4, in_=X4[:, :, ::-1, :])
                nc.vector.copy_predicated(X4, MV[:].rearrange(
                    "p (q c) -> p q c", c=16).rearrange("p q (i j) -> p q i j", i=4), Bv4)
                nc.gpsimd.tensor_copy(out=Ch4, in_=X4[:, :, :, ::-1])
                nc.vector.copy_predicated(X4, MH[:].rearrange(
                    "p (q c) -> p q c", c=16).rearrange("p q (i j) -> p q i j", i=4), Ch4)

                XB = bpool.tile([128, 1024], BF16, tag="XB")
                nc.gpsimd.tensor_copy(out=XB[:], in_=X[:])

                ETT = bpool.tile([128, 1024], F16, tag="ETT")

                # ============================== chunks of 1024 samples
                for k in range(NCHUNK):
                    Y = kpool.tile([128, 128], BF16, tag="Y")
                    nc.sync.dma_start_transpose(Y[:], XB[:, 128 * k:128 * (k + 1)])
                    # one-hot rows replicated from Y cell rows on the PE:
                    # EEp[(c,v), 128*qt + p] = e[cell] of sample (qt, p);
                    # shared double-buffered [88,512] psum, consumed by the
                    # is_equal evacuation immediately
                    OH0 = kpool.tile([88, 1024], BF16, tag="OH0")
                    OH1 = kpool.tile([88, 1024], BF16, tag="OH1")
                    for hh in range(2):
                        for cell8, oh in ((0, OH0), (1, OH1)):
                            eep = ppool2.tile([88, 512], F32, tag="EEp")
                            for qq in range(4):
                                qt = 4 * hh + qq
                                i = 2 * qt + cell8
                                nc.tensor.matmul(
                                    eep[:, 128 * qq:128 * (qq + 1)],
                                    Rep[:, 88 * i:88 * (i + 1)], Y[:],
                                    start=True, stop=True)
                            nc.vector.tensor_scalar(
                                out=oh[:, 512 * hh:512 * (hh + 1)], in0=eep[:],
                                scalar1=Vc[:], scalar2=None, op0=AOP.is_equal)

                    for h in range(2):
                        sl = slice(512 * h, 512 * (h + 1))
                        z1p = ppool2.tile([96, 512], F32, tag="z1p")
                        nc.tensor.matmul(z1p[:], G0[:], OH0[:, sl], start=True, stop=False)
                        nc.tensor.matmul(z1p[:], G1[:], OH1[:, sl], start=False, stop=True)
                        z1 = mpool.tile([96, 512], BF16, tag="z1")
                        nc.vector.tensor_scalar(out=z1[:], in0=z1p[:], scalar1=b1c[:],
                                                scalar2=0.0, op0=AOP.add, op1=AOP.max)

                        z2pa = ppool.tile([128, 512], F32, tag="z2pa")
                        z2pb = ppool.tile([128, 512], F32, tag="z2pb")
                        nc.tensor.matmul(z2pa[:], W1a[:], z1[:], start=True, stop=True)
                        nc.tensor.matmul(z2pb[:], W1b[:], z1[:], start=True, stop=True)
                        z2a = mpool.tile([128, 512], BF16, tag="z2a")
                        z2b = mpool.tile([128, 512], BF16, tag="z2b")
                        nc.scalar.activation(z2a[:], z2pa[:], AF.Relu, bias=b2ac[:])
                        nc.scalar.activation(z2b[:], z2pb[:], AF.Relu, bias=b2bc[:])

                        # L3 computed sample-major: z2 slices are the
                        # stationary operand, W2 the moving one:
                        # LgT[p, 16b + r] = logit r of sample 128b + p
                        LgT = ppool2.tile([128, 64], F32, tag="LgT")
                        for b in range(4):
                            nc.tensor.matmul(LgT[:, 16 * b:16 * b + 16],
                                             z2a[:, 128 * b:128 * (b + 1)], W2a[:],
                                             start=True, stop=False)
                            nc.tensor.matmul(LgT[:, 16 * b:16 * b + 16],
                                             z2b[:, 128 * b:128 * (b + 1)], W2b[:],
                                             start=False, stop=True)
                        # exp straight into sample-order columns of ETT:
                        # LgT col 16b+r = variant r of q = 8k + 4h + b
                        q0 = 8 * k + 4 * h
                        nc.scalar.activation(ETT[:, 16 * q0:16 * q0 + 64], LgT[:],
                                             AF.Exp, bias=bsc[:])
                        # non-constant out_b correction (all-ones otherwise)
                        nc.gpsimd.tensor_tensor(
                            out=ETT[:, 16 * q0:16 * q0 + 64],
                            in0=ETT[:, 16 * q0:16 * q0 + 64], in1=EBc[:],
                            op=AOP.mult)

                # ============================== stage D: softmax+select (sample-major)
                # ETT[p, 16q + 4f + j] = exp(variant-f logit j) of sample 64p+q
                E4 = ETT[:].rearrange("p (g f j) -> p g f j", f=4, j=4)
                T1 = bpool.tile([128, 256], F32, tag="T1")
                T2 = bpool.tile([128, 256], F32, tag="T2")
                S4 = bpool.tile([128, 256], F32, tag="S4")
                T1r = T1[:].rearrange("p (g f) -> p g f", f=4)
                T2r = T2[:].rearrange("p (g f) -> p g f", f=4)
                S4r = S4[:].rearrange("p (g f) -> p g f", f=4)
                nc.vector.tensor_tensor(out=T1r, in0=E4[:, :, :, 0], in1=E4[:, :, :, 1], op=AOP.add)
                nc.vector.tensor_tensor(out=T2r, in0=E4[:, :, :, 2], in1=E4[:, :, :, 3], op=AOP.add)
                nc.vector.tensor_tensor(out=S4r, in0=T1r, in1=T2r, op=AOP.add)

                # masks are already in sample (q) order
                FVE, FHE = FV, FH
                FBE = bpool.tile([128, 64], I32, tag="FBE")
                nc.vector.tensor_tensor(out=FBE[:], in0=FVE[:], in1=FHE[:], op=AOP.mult)

                Ssel = bpool.tile([128, 64], F32, tag="Ssel")
                nc.vector.tensor_copy(out=Ssel[:], in_=S4r[:, :, 0])
                nc.vector.copy_predicated(Ssel[:], FHE[:], S4r[:, :, 1])
                nc.vector.copy_predicated(Ssel[:], FVE[:], S4r[:, :, 2])
                nc.vector.copy_predicated(Ssel[:], FBE[:], S4r[:, :, 3])
                RS = bpool.tile([128, 64], F32, tag="RS")
                nc.vector.reciprocal(RS[:], Ssel[:])

                # padded to stride 5 so the 3D group view cannot be flattened
                # (sim requires identically-shaped APs in copy_predicated)
                Esel = bpool.tile([128, 320], F16, tag="Esel")
                Er = Esel[:].rearrange("p (g j) -> p g j", j=5)[:, :, 0:4]
                nc.vector.tensor_copy(out=Er, in_=E4[:, :, 0, :])
                nc.vector.copy_predicated(
                    Er, FHE[:].unsqueeze(2).broadcast_to([128, 64, 4]), E4[:, :, 1, :])
                nc.vector.copy_predicated(
                    Er, FVE[:].unsqueeze(2).broadcast_to([128, 64, 4]), E4[:, :, 2, :])
                nc.vector.copy_predicated(
                    Er, FBE[:].unsqueeze(2).broadcast_to([128, 64, 4]), E4[:, :, 3, :])

                PR = bpool.tile([128, 320], F32, tag="PR")
                PRr = PR[:].rearrange("p (g j) -> p g j", j=5)[:, :, 0:4]
                nc.vector.tensor_tensor(
                    out=PRr, in0=Er,
                    in1=RS[:].unsqueeze(2).broadcast_to([128, 64, 4]), op=AOP.mult)

                nc.sync.dma_start(out=outv[n], in_=PRr)
    return nc


def _compile():
    global _COMPILED
    if _COMPILED is None:
        import concourse.bacc as bacc
        nc = bacc.Bacc("TRN2", target_bir_lowering=False)
        build_ir(nc)
        nc.compile()
        _COMPILED = nc
    return _COMPILED


# ----------------------------------------------------------------- entry
def kernel(**inputs):
    from concourse.bass_utils import run_bass_kernel_spmd
    nc = _compile()
    consts = _make_consts(inputs)
    e = np.ascontiguousarray(np.asarray(inputs["exponents"], np.int32))
    in_maps = []
    for i in range(N_CORES):
        m = dict(consts)
        m["exponents"] = np.ascontiguousarray(e[i * S:(i + 1) * S])
        in_maps.append(m)
    res = run_bass_kernel_spmd(nc, in_maps, core_ids=list(range(N_CORES)))
    return np.concatenate([res.results[i]["out"] for i in range(N_CORES)], axis=0)


# revision 21
# speedup vs baseline: 1.3744x; 1.3744x over previous
"""Trainium2 Bass kernel for nn_SmartCNN (2048-style board CNN policy net).

Algorithm (per sample, 4x4 board of exponents e in [0,11)):
  1. flips (fv, fh) from corner argmax of the board (first-max-wins),
     applied as data flips to the int exponents (sample-major, DVE selects).
  2. one-hot(176) of flipped exponents built feature-major:
     bf16 cast -> DMA-xbar transpose -> DMA partition-replication x11 ->
     is_equal against per-partition class id.
  3. z1 = relu(G @ oh + b1)   [96]   (G fuses both convs + the constant
     mask channel; computed on host from conv weights)
  4. z2 = relu(W1 @ z1 + b2)  [256]
  5. Lg = W2x4 @ z2 + b3x4    [16]   (4 flip-variant-permuted copies of the
     output layer -> per-sample variant selected after softmax transpose)
  6. out = softmax(Lg[4f:4f+4]), f = 2*fv+fh; computed sample-major after a
     DMA transpose of exp(Lg - SHIFT).

Data parallel over 8 NeuronCores: each core does 32768 samples.
"""
import numpy as np
import ml_dtypes

B_TOTAL = 262144
N_CORES = 8
S = B_TOTAL // N_CORES          # 32768 samples per core
BIG = 8192                      # samples per big tile (sample-major stages)
NBIG = S // BIG                 # 4
NCHUNK = BIG // 1024            # 8 transpose chunks per big tile
CLASSES = 11

_COMPILED = None  # cached (nc, const_inputs) across calls


# ----------------------------------------------------------------- host math
def _build_weights(conv0_w, conv0_b, conv1_w, linear_w, linear_b, out_w, out_b):
    G = np.zeros((96, 176), np.float32)
    b1 = np.zeros((96,), np.float32)
    for oc in range(4):
        for i in range(4):
            for jj in range(3):
                k = oc * 12 + i * 3 + jj
                b1[k] = conv0_b[oc] + conv0_w[oc, 0, 0, 0] + conv0_w[oc, 0, 0, 1]
                for kx in range(2):
                    c = i * 4 + (jj + kx)
                    G[k, c * 11:(c + 1) * 11] += conv0_w[oc, 1:, 0, kx]
    for oc in range(4):
        for ii in range(3):
            for j in range(4):
                k = 48 + oc * 12 + ii * 4 + j
                b1[k] = conv1_w[oc, 0, 0, 0] + conv1_w[oc, 0, 1, 0]
                for ky in range(2):
                    c = (ii + ky) * 4 + j
                    G[k, c * 11:(c + 1) * 11] += conv1_w[oc, 1:, ky, 0]
    W2x4 = np.zeros((16, 256), np.float32)
    b3x4 = np.zeros((16,), np.float32)
    for fv in range(2):
        for fh in range(2):
            f = 2 * fv + fh
            perm = [1 if fv else 0, 0 if fv else 1, 3 if fh else 2, 2 if fh else 3]
            for j in range(4):
                W2x4[4 * f + j] = out_w[perm[j]]
                b3x4[4 * f + j] = out_b[perm[j]]
    # softmax shift: tight data-independent upper bound on the logits
    z1_hi = np.maximum(
        b1 + G.reshape(96, 16, 11).max(2).sum(1) - 0.0, 0)  # only 2 cells/k are nonzero; sum of per-cell maxes is a valid bound
    z2_hi = np.maximum(linear_b + np.maximum(linear_w, 0) @ z1_hi, 0)
    lg_hi = (b3x4 + np.maximum(W2x4, 0) @ z2_hi).max()
    shift = float(lg_hi) - 8.0
    return (G.astype(np.float32), b1.astype(np.float32),
            np.asarray(linear_w, np.float32), np.asarray(linear_b, np.float32),
            W2x4, b3x4, shift)


def _make_consts(inputs):
    G, b1, W1, b2, W2x4, b3x4, shift = _build_weights(
        inputs["conv0_w"], inputs["conv0_b"], inputs["conv1_w"],
        inputs["linear_w"], inputs["linear_b"], inputs["out_w"], inputs["out_b"])
    bf = ml_dtypes.bfloat16
    return {
        "G0c": np.ascontiguousarray(G[:, 0:88].T).astype(bf),       # [88,96]
        "G1c": np.ascontiguousarray(G[:, 88:176].T).astype(bf),     # [88,96]
        "b1col": b1.reshape(96, 1),
        "W1a": np.ascontiguousarray(W1[0:128].T).astype(bf),        # [96,128]
        "W1b": np.ascontiguousarray(W1[128:256].T).astype(bf),      # [96,128]
        "b2a": b2[0:128].reshape(128, 1).astype(np.float32),
        "b2b": b2[128:256].reshape(128, 1).astype(np.float32),
        "W2a": np.ascontiguousarray(W2x4[:, 0:128].T).astype(bf),   # [128,16]
        "W2b": np.ascontiguousarray(W2x4[:, 128:256].T).astype(bf), # [128,16]
        "Vcol": (np.arange(88, dtype=np.float32) % 11).reshape(88, 1),
        # sample-major softmax: constant part of the logit bias as an
        # all-equal [128,1] exp-bias column; per-column remainder as a
        # broadcast multiplier tile (all-ones when out_b is constant).
        "bshift": np.full((128, 1), b3x4[0] - shift, np.float32),
        "EB64": np.tile(np.exp(b3x4 - b3x4[0]).astype(np.float32), (128, 4)),
        "RepAll": _rep_matrices(),
    }


def _rep_matrices():
    """16 stationary one-hot broadcasters as column blocks of [128, 1408]:
    block i = (qt, half): (Rep_i.T @ Y)[(c,v), p] = Y[16qt + 8*half + c, p]."""
    rep = np.zeros((128, 16, 88), np.float32)
    for qt in range(8):
        for half in range(2):
            for c in range(8):
                for v in range(11):
                    rep[16 * qt + 8 * half + c, 2 * qt + half, c * 11 + v] = 1.0
    return rep.reshape(128, 1408).astype(ml_dtypes.bfloat16)


# ----------------------------------------------------------------- device IR
def build_ir(nc, samples=S):
    import concourse.mybir as mybir
    import concourse.tile as tile

    F32, I32 = mybir.dt.float32, mybir.dt.int32
    BF16, F16 = mybir.dt.bfloat16, mybir.dt.float16
    AOP = mybir.AluOpType
    AF = mybir.ActivationFunctionType

    nbig = samples // BIG

    ex = nc.dram_tensor("exponents", [samples, 16], I32, kind="ExternalInput")
    out = nc.dram_tensor("out", [samples, 4], F32, kind="ExternalOutput")
    cG0 = nc.dram_tensor("G0c", [88, 96], BF16, kind="ExternalInput")
    cG1 = nc.dram_tensor("G1c", [88, 96], BF16, kind="ExternalInput")
    cb1 = nc.dram_tensor("b1col", [96, 1], F32, kind="ExternalInput")
    cW1a = nc.dram_tensor("W1a", [96, 128], BF16, kind="ExternalInput")
    cW1b = nc.dram_tensor("W1b", [96, 128], BF16, kind="ExternalInput")
    cb2a = nc.dram_tensor("b2a", [128, 1], F32, kind="ExternalInput")
    cb2b = nc.dram_tensor("b2b", [128, 1], F32, kind="ExternalInput")
    cW2a = nc.dram_tensor("W2a", [128, 16], BF16, kind="ExternalInput")
    cW2b = nc.dram_tensor("W2b", [128, 16], BF16, kind="ExternalInput")
    cV = nc.dram_tensor("Vcol", [88, 1], F32, kind="ExternalInput")
    cbs = nc.dram_tensor("bshift", [128, 1], F32, kind="ExternalInput")
    cEB = nc.dram_tensor("EB64", [128, 64], F32, kind="ExternalInput")
    cRep = nc.dram_tensor("RepAll", [128, 1408], BF16, kind="ExternalInput")

    with tile.TileContext(nc) as tc:
        with (
            tc.tile_pool(name="const", bufs=1) as cpool,
            tc.tile_pool(name="big", bufs=2) as bpool,
            tc.tile_pool(name="chunk", bufs=3) as kpool,
            tc.tile_pool(name="mid", bufs=3) as mpool,
            tc.tile_pool(name="ps", bufs=1, space="PSUM") as ppool,
            tc.tile_pool(name="ps2", bufs=2, space="PSUM") as ppool2,
        ):
            # ---- load constants once
            G0 = cpool.tile([88, 96], BF16, tag="G0")
            G1 = cpool.tile([88, 96], BF16, tag="G1")
            b1c = cpool.tile([96, 1], F32, tag="b1c")
            W1a = cpool.tile([96, 128], BF16, tag="W1a")
            W1b = cpool.tile([96, 128], BF16, tag="W1b")
            b2ac = cpool.tile([128, 1], F32, tag="b2a")
            b2bc = cpool.tile([128, 1], F32, tag="b2b")
            W2a = cpool.tile([128, 16], BF16, tag="W2a")
            W2b = cpool.tile([128, 16], BF16, tag="W2b")
            Vc = cpool.tile([88, 1], F32, tag="Vc")
            bsc = cpool.tile([128, 1], F32, tag="bsc")
            EBc = cpool.tile([128, 64], F32, tag="EBc")
            Rep = cpool.tile([128, 1408], BF16, tag="Rep")
            for t, d in ((G0, cG0), (G1, cG1), (b1c, cb1), (W1a, cW1a),
                         (W1b, cW1b), (b2ac, cb2a), (b2bc, cb2b), (W2a, cW2a),
                         (W2b, cW2b), (Vc, cV), (bsc, cbs), (EBc, cEB),
                         (Rep, cRep)):
                nc.sync.dma_start(out=t[:], in_=d[:])

            exv = ex[:].rearrange("(n p q) c -> n p (q c)", p=128, q=64)
            outv = out[:].rearrange("(n p q) j -> n p (q j)", p=128, q=64)

            for n in range(nbig):
                # ============================== stage A: flips (sample-major)
                X = bpool.tile([128, 1024], I32, tag="X")
                nc.sync.dma_start(out=X[:], in_=exv[n])
                X4 = X[:].rearrange("p (q i j) -> p q i j", i=4, j=4)
                c0, c3 = X4[:, :, 0, 0], X4[:, :, 0, 3]
                c12, c15 = X4[:, :, 3, 0], X4[:, :, 3, 3]
                m01 = bpool.tile([128, 64], I32, tag="m01")
                m23 = bpool.tile([128, 64], I32, tag="m23")
                FV = bpool.tile([128, 64], I32, tag="FV")
                FH = bpool.tile([128, 64], I32, tag="FH")
                H1 = bpool.tile([128, 64], I32, tag="H1")
                nc.vector.tensor_tensor(out=m01[:], in0=c0, in1=c3, op=AOP.max)
                nc.vector.tensor_tensor(out=m23[:], in0=c12, in1=c15, op=AOP.max)
                nc.vector.tensor_tensor(out=FV[:], in0=m23[:], in1=m01[:], op=AOP.is_gt)
                nc.vector.tensor_tensor(out=FH[:], in0=c3, in1=c0, op=AOP.is_gt)
                nc.vector.tensor_tensor(out=H1[:], in0=c15, in1=c12, op=AOP.is_gt)
                nc.vector.copy_predicated(FH[:], FV[:], H1[:])

                MV = bpool.tile([128, 1024], I32, tag="MV")
                MH = bpool.tile([128, 1024], I32, tag="MH")
                nc.vector.tensor_copy(
                    out=MV[:].rearrange("p (q c) -> p q c", c=16),
                    in_=FV[:].unsqueeze(2).broadcast_to([128, 64, 16]))
                nc.vector.tensor_copy(
                    out=MH[:].rearrange("p (q c) -> p q c", c=16),
                    in_=FH[:].unsqueeze(2).broadcast_to([128, 64, 16]))

                Bv = bpool.tile([128, 1024], I32, tag="Bv")
                Ch = bpool.tile([128, 1024], I32, tag="Ch")
                Bv4 = Bv[:].rearrange("p (q i j) -> p q i j", i=4, j=4)
                Ch4 = Ch[:].rearrange("p (q i j) -> p q i j", i=4, j=4)
                nc.gpsimd.tensor_copy(out=Bv4, in_=X4[:, :, ::-1, :])
                nc.vector.copy_predicated(X4, MV[:].rearrange(
                    "p (q c) -> p q c", c=16).rearrange("p q (i j) -> p q i j", i=4), Bv4)
                nc.gpsimd.tensor_copy(out=Ch4, in_=X4[:, :, :, ::-1])
                nc.vector.copy_predicated(X4, MH[:].rearrange(
                    "p (q c) -> p q c", c=16).rearrange("p q (i j) -> p q i j", i=4), Ch4)

                XB = bpool.tile([128, 1024], BF16, tag="XB")
                nc.scalar.copy(out=XB[:], in_=X[:])

                ETT = bpool.tile([128, 1024], F16, tag="ETT")

                # ============================== chunks of 1024 samples
                for k in range(NCHUNK):
                    Y = kpool.tile([128, 128], BF16, tag="Y")
                    nc.sync.dma_start_transpose(Y[:], XB[:, 128 * k:128 * (k + 1)])
                    # one-hot rows replicated from Y cell rows on the PE:
                    # EEp[(c,v), 128*qt + p] = e[cell] of sample (qt, p);
                    # shared double-buffered psum tag so chunk t+1's rep
                    # matmuls overlap chunk t's is_equal evacuation
                    OH0 = kpool.tile([88, 1024], BF16, tag="OH0")
                    OH1 = kpool.tile([88, 1024], BF16, tag="OH1")
                    for cell8, oh in ((0, OH0), (1, OH1)):
                        eep = ppool2.tile([88, 1024], F32, tag="EEp")
                        for qt in range(8):
                            i = 2 * qt + cell8
                            nc.tensor.matmul(eep[:, 128 * qt:128 * (qt + 1)],
                                             Rep[:, 88 * i:88 * (i + 1)], Y[:],
                                             start=True, stop=True)
                        nc.vector.tensor_scalar(out=oh[:], in0=eep[:], scalar1=Vc[:],
                                                scalar2=None, op0=AOP.is_equal)

                    for h in range(2):
                        sl = slice(512 * h, 512 * (h + 1))
                        z1p = ppool.tile([96, 512], F32, tag="z1p")
                        nc.tensor.matmul(z1p[:], G0[:], OH0[:, sl], start=True, stop=False)
                        nc.tensor.matmul(z1p[:], G1[:], OH1[:, sl], start=False, stop=True)
                        z1 = mpool.tile([96, 512], BF16, tag="z1")
                        nc.scalar.activation(z1[:], z1p[:], AF.Relu, bias=b1c[:])

                        z2pa = ppool.tile([128, 512], F32, tag="z2pa")
                        z2pb = ppool.tile([128, 512], F32, tag="z2pb")
                        nc.tensor.matmul(z2pa[:], W1a[:], z1[:], start=True, stop=True)
                        nc.tensor.matmul(z2pb[:], W1b[:], z1[:], start=True, stop=True)
                        z2a = mpool.tile([128, 512], BF16, tag="z2a")
                        z2b = mpool.tile([128, 512], BF16, tag="z2b")
                        nc.scalar.activation(z2a[:], z2pa[:], AF.Relu, bias=b2ac[:])
                        nc.vector.tensor_scalar(out=z2b[:], in0=z2pb[:], scalar1=b2bc[:],
                                                scalar2=0.0, op0=AOP.add, op1=AOP.max)

                        # L3 computed sample-major: z2 slices are the
                        # stationary operand, W2 the moving one:
                        # LgT[p, 16b + r] = logit r of sample 128b + p
                        LgT = ppool.tile([128, 64], F32, tag="LgT")
                        for b in range(4):
                            nc.tensor.matmul(LgT[:, 16 * b:16 * b + 16],
                                             z2a[:, 128 * b:128 * (b + 1)], W2a[:],
                                             start=True, stop=False)
                            nc.tensor.matmul(LgT[:, 16 * b:16 * b + 16],
                                             z2b[:, 128 * b:128 * (b + 1)], W2b[:],
                                             start=False, stop=True)
                        # exp straight into sample-order columns of ETT:
                        # LgT col 16b+r = variant r of q = 8k + 4h + b
                        q0 = 8 * k + 4 * h
                        nc.scalar.activation(ETT[:, 16 * q0:16 * q0 + 64], LgT[:],
                                             AF.Exp, bias=bsc[:])
                        # non-constant out_b correction (all-ones otherwise)
                        nc.gpsimd.tensor_tensor(
                            out=ETT[:, 16 * q0:16 * q0 + 64],
                            in0=ETT[:, 16 * q0:16 * q0 + 64], in1=EBc[:],
                            op=AOP.mult)

                # ============================== stage D: softmax+select (sample-major)
                # ETT[p, 16q + 4f + j] = exp(variant-f logit j) of sample 64p+q
                E4 = ETT[:].rearrange("p (g f j) -> p g f j", f=4, j=4)
                T1 = bpool.tile([128, 256], F32, tag="T1")
                T2 = bpool.tile([128, 256], F32, tag="T2")
                S4 = bpool.tile([128, 256], F32, tag="S4")
                T1r = T1[:].rearrange("p (g f) -> p g f", f=4)
                T2r = T2[:].rearrange("p (g f) -> p g f", f=4)
                S4r = S4[:].rearrange("p (g f) -> p g f", f=4)
                nc.vector.tensor_tensor(out=T1r, in0=E4[:, :, :, 0], in1=E4[:, :, :, 1], op=AOP.add)
                nc.vector.tensor_tensor(out=T2r, in0=E4[:, :, :, 2], in1=E4[:, :, :, 3], op=AOP.add)
                nc.vector.tensor_tensor(out=S4r, in0=T1r, in1=T2r, op=AOP.add)

                # masks are already in sample (q) order
                FVE, FHE = FV, FH
                FBE = bpool.tile([128, 64], I32, tag="FBE")
                nc.vector.tensor_tensor(out=FBE[:], in0=FVE[:], in1=FHE[:], op=AOP.mult)

                Ssel = bpool.tile([128, 64], F32, tag="Ssel")
                nc.vector.tensor_copy(out=Ssel[:], in_=S4r[:, :, 0])
                nc.vector.copy_predicated(Ssel[:], FHE[:], S4r[:, :, 1])
                nc.vector.copy_predicated(Ssel[:], FVE[:], S4r[:, :, 2])
                nc.vector.copy_predicated(Ssel[:], FBE[:], S4r[:, :, 3])
                RS = bpool.tile([128, 64], F32, tag="RS")
                nc.vector.reciprocal(RS[:], Ssel[:])

                # padded to stride 5 so the 3D group view cannot be flattened
                # (sim requires identically-shaped APs in copy_predicated)
                Esel = bpool.tile([128, 320], F16, tag="Esel")
                Er = Esel[:].rearrange("p (g j) -> p g j", j=5)[:, :, 0:4]
                nc.vector.tensor_copy(out=Er, in_=E4[:, :, 0, :])
                nc.vector.copy_predicated(
                    Er, FHE[:].unsqueeze(2).broadcast_to([128, 64, 4]), E4[:, :, 1, :])
                nc.vector.copy_predicated(
                    Er, FVE[:].unsqueeze(2).broadcast_to([128, 64, 4]), E4[:, :, 2, :])
                nc.vector.copy_predicated(
                    Er, FBE[:].unsqueeze(2).broadcast_to([128, 64, 4]), E4[:, :, 3, :])

                PR = bpool.tile([128, 320], F32, tag="PR")
                PRr = PR[:].rearrange("p (g j) -> p g j", j=5)[:, :, 0:4]
                nc.vector.tensor_tensor(
                    out=PRr, in0=Er,
                    in1=RS[:].unsqueeze(2).broadcast_to([128, 64, 4]), op=AOP.mult)

                nc.sync.dma_start(out=outv[n], in_=PRr)
    return nc


def _compile():
    global _COMPILED
    if _COMPILED is None:
        import concourse.bacc as bacc
        nc = bacc.Bacc("TRN2", target_bir_lowering=False)
        build_ir(nc)
        nc.compile()
        _COMPILED = nc
    return _COMPILED


# ----------------------------------------------------------------- entry
def kernel(**inputs):
    from concourse.bass_utils import run_bass_kernel_spmd
    nc = _compile()
    consts = _make_consts(inputs)
    e = np.ascontiguousarray(np.asarray(inputs["exponents"], np.int32))
    in_maps = []
    for i in range(N_CORES):
        m = dict(consts)
        m["exponents"] = np.ascontiguousarray(e[i * S:(i + 1) * S])
        in_maps.append(m)
    res = run_bass_kernel_spmd(nc, in_maps, core_ids=list(range(N_CORES)))
    return np.concatenate([res.results[i]["out"] for i in range(N_CORES)], axis=0)
